# revision 5
# baseline (speedup 1.0000x reference)
"""Distributed Trainium2 kernel for the ABS-MAE partial-label loss.

Math: for p = softmax(outputs, axis=1) and eye the CxC identity,
    sum_k |p[n,k] - eye[j,k]| = (1 - p[n,j]) + |p[n,j] - 1| = 2 - 2*p[n,j]
so with conf = label_confidence[index] (rows of conf sum to 1),
    loss_mean = (1/N) * sum_n sum_j conf[n,j] * (2 - 2*p[n,j])
              = 2 - (2/N) * sum_n <p[n], conf[n]>.

Sharding (8 cores): label_confidence is row-sharded (6250 rows/core) and
the batch is sharded by ownership — core c handles exactly the batch items
whose index falls in its table shard (padded to K slots), so no cross-core
row movement is needed.

Device layout: each logical row is split into Q = 128/K quarters so all 128
SBUF partitions are busy (DVE/ACT time scales with free-dim length). The
conf gather uses the table viewed as [ROWS*Q, C/Q] with host-expanded
quarter indices. Per-partition exp row-sums / row-dots are group-summed
back to K logical rows with a one-hot selection matmul on the TensorEngine,
and the final masked, normalized reduction is two more tiny matmuls.
Each core outputs  out_c = 2/8 - (2/N) * sum_own <p, conf>;  unsharding
sums the 8 partials.
"""

import numpy as np

import concourse.bass as bass
import concourse.bacc as bacc
import concourse.mybir as mybir
import concourse.tile as tile
from concourse.bass_utils import run_bass_kernel_spmd

N = 128          # batch
C = 1000         # classes
NUM_DATA = 50000 # table rows
CORES = 8
ROWS = NUM_DATA // CORES  # 6250 per-core table shard
P = 128          # SBUF partitions

_nc_cache = {}
LAST_RESULTS = None  # BassKernelResults from the most recent run (for test harness)


def _build(K):
    Q = P // K        # quarters per logical row
    CQ = C // Q       # columns per quarter
    W = CQ + 1 + K    # packed input: [x_q | mask | sel]
    f32 = mybir.dt.float32
    i32 = mybir.dt.int32
    nc = bacc.Bacc(
        "TRN2", target_bir_lowering=False, debug=False, num_devices=CORES
    )

    xall_ext = nc.dram_tensor("xall", [P, W], f32, kind="ExternalInput")
    t_ext = nc.dram_tensor("table", [ROWS * Q, CQ], f32, kind="ExternalInput")
    gidx_ext = nc.dram_tensor("gidx", [P, 1], i32, kind="ExternalInput")
    out_ext = nc.dram_tensor("out", [1, 1], f32, kind="ExternalOutput")

    with tile.TileContext(nc) as tc:
        with (
            tc.tile_pool(name="sbuf", bufs=1) as sb,
            tc.tile_pool(name="psum", bufs=1, space="PSUM") as ps,
        ):
            # warm the ACT exp table while input DMAs are in flight
            warm = sb.tile([1, 1], f32)
            nc.vector.memset(warm[:], 0.0)
            warm2 = sb.tile([1, 1], f32)
            nc.scalar.activation(
                out=warm2[:], in_=warm[:], func=mybir.ActivationFunctionType.Exp
            )

            # ---- loads: one packed DMA on sync, gather index on gpsimd ----
            xall = sb.tile([P, W], f32)
            nc.sync.dma_start(out=xall[:], in_=xall_ext[:])
            gidx = sb.tile([P, 1], i32)
            nc.gpsimd.dma_start(out=gidx[:], in_=gidx_ext[:])

            x = xall[:, 0:CQ]
            maskv = xall[0:K, CQ : CQ + 1]           # pre-scaled by -2/N
            sel = xall[:, CQ + 1 : CQ + 1 + K]       # one-hot group matrix

            # ---- gather conf quarter-rows for the owned batch items ----
            conf = sb.tile([P, CQ], f32)
            nc.gpsimd.indirect_dma_start(
                out=conf[:],
                out_offset=None,
                in_=t_ext[:],
                in_offset=bass.IndirectOffsetOnAxis(ap=gidx[:, :1], axis=0),
            )

            # ---- e = exp(x) with per-partition sums (x ~ N(0,1): no shift) ----
            e = sb.tile([P, CQ], f32)
            s4 = sb.tile([P, 1], f32)
            nc.scalar.activation(
                out=e[:],
                in_=x,
                func=mybir.ActivationFunctionType.Exp,
                bias=0.0,
                scale=1.0,
                accum_out=s4[:],
            )

            # ---- group-sum sumexp to K logical rows; w = mask / sumexp ----
            s32p = ps.tile([K, 1], f32)
            nc.tensor.matmul(out=s32p[:], lhsT=sel, rhs=s4[:], start=True, stop=True)
            recip = sb.tile([K, 1], f32)
            nc.vector.reciprocal(out=recip[:], in_=s32p[:])
            w32 = sb.tile([K, 1], f32)
            nc.vector.tensor_mul(w32[:], maskv, recip[:])

            # ---- per-partition rowdot, group-sum, masked total ----
            prod = sb.tile([P, CQ], f32)
            nc.vector.tensor_mul(prod[:], e[:], conf[:])
            r4 = sb.tile([P, 1], f32)
            nc.vector.reduce_sum(out=r4[:], in_=prod[:], axis=mybir.AxisListType.X)
            r32p = ps.tile([K, 1], f32)
            nc.tensor.matmul(out=r32p[:], lhsT=sel, rhs=r4[:], start=True, stop=True)
            r32 = sb.tile([K, 1], f32)
            nc.vector.tensor_copy(out=r32[:], in_=r32p[:])
            acc = ps.tile([1, 1], f32)
            nc.tensor.matmul(out=acc[:], lhsT=r32[:], rhs=w32[:], start=True, stop=True)

            # ---- out_c = partial + 2/CORES ----
            final = sb.tile([1, 1], f32)
            nc.scalar.activation(
                out=final[:],
                in_=acc[:],
                func=mybir.ActivationFunctionType.Copy,
                bias=2.0 / CORES,
                scale=1.0,
            )
            nc.sync.dma_start(out=out_ext[:], in_=final[:])

    nc.compile()
    return nc


def _get_nc(K):
    if K not in _nc_cache:
        _nc_cache[K] = _build(K)
    return _nc_cache[K]


def kernel(outputs, label_confidence, index):
    global LAST_RESULTS
    outputs = np.ascontiguousarray(np.asarray(outputs, dtype=np.float32))
    label_confidence = np.ascontiguousarray(
        np.asarray(label_confidence, dtype=np.float32)
    )
    idx = np.asarray(index).astype(np.int64).reshape(N)

    owner = idx // ROWS
    counts = np.bincount(owner, minlength=CORES)
    K = 32
    while K < int(counts.max()):
        K *= 2
    Q = P // K
    CQ = C // Q
    W = CQ + 1 + K
    nc = _get_nc(K)

    sel = np.zeros((P, K), dtype=np.float32)
    sel[np.arange(P), np.arange(P) // Q] = 1.0

    in_maps = []
    for c in range(CORES):
        rows = np.nonzero(owner == c)[0]
        n_own = len(rows)
        rows_p = np.concatenate([rows, np.zeros(K - n_own, dtype=rows.dtype)])
        # quarter-expanded gather indices into the [ROWS*Q, CQ] table view
        g = (idx[rows_p] - c * ROWS).astype(np.int64)
        g[n_own:] = 0
        gidx = (g[:, None] * Q + np.arange(Q)[None, :]).astype(np.int32).reshape(P, 1)
        mask = np.full(K, -2.0 / N, dtype=np.float32)
        mask[n_own:] = 0.0

        xall = np.empty((P, W), dtype=np.float32)
        xall[:, 0:CQ] = outputs[rows_p].reshape(P, CQ)
        xall[:, CQ] = 0.0
        xall[0:K, CQ] = mask
        xall[:, CQ + 1 :] = sel
        in_maps.append(
            {
                "xall": xall,
                "table": label_confidence[c * ROWS : (c + 1) * ROWS].reshape(
                    ROWS * Q, CQ
                ),
                "gidx": gidx,
            }
        )
    LAST_RESULTS = run_bass_kernel_spmd(nc, in_maps, core_ids=list(range(CORES)))
    total = np.float32(0.0)
    for c in range(CORES):
        total += np.float32(LAST_RESULTS.results[c]["out"][0, 0])
    return np.asarray(total, dtype=np.float32).reshape(())


# revision 8
# speedup vs baseline: 1.1044x; 1.1044x over previous
"""Distributed Trainium2 kernel for the ABS-MAE partial-label loss.

Math: for p = softmax(outputs, axis=1) and eye the CxC identity,
    sum_k |p[n,k] - eye[j,k]| = (1 - p[n,j]) + |p[n,j] - 1| = 2 - 2*p[n,j]
so with conf = label_confidence[index] (rows of conf sum to 1),
    loss_mean = (1/N) * sum_n sum_j conf[n,j] * (2 - 2*p[n,j])
              = 2 - (2/N) * sum_n <p[n], conf[n]>.

Sharding (8 cores): label_confidence is row-sharded (6250 rows/core) and
the batch is sharded by ownership — core c handles exactly the batch items
whose index falls in its table shard (padded to K slots), so no cross-core
row movement is needed.

Device layout: each logical row is split into Q = 128/K quarters so all 128
SBUF partitions are busy (DVE/ACT time scales with free-dim length). The
conf gather uses the table viewed as [ROWS*Q, C/Q] with host-expanded
quarter indices. Per-partition exp row-sums / row-dots are group-summed
back to K logical rows with a one-hot selection matmul on the TensorEngine,
and the final masked, normalized reduction is two more tiny matmuls.
Each core outputs  out_c = 2/8 - (2/N) * sum_own <p, conf>;  unsharding
sums the 8 partials.
"""

import numpy as np

import concourse.bass as bass
import concourse.bacc as bacc
import concourse.mybir as mybir
import concourse.tile as tile
from concourse.bass_utils import run_bass_kernel_spmd

N = 128          # batch
C = 1000         # classes
NUM_DATA = 50000 # table rows
CORES = 8
ROWS = NUM_DATA // CORES  # 6250 per-core table shard
P = 128          # SBUF partitions

_nc_cache = {}
LAST_RESULTS = None  # BassKernelResults from the most recent run (for test harness)


def _build(K):
    Q = P // K        # quarters per logical row
    CQ = C // Q       # columns per quarter
    W = CQ + 1 + K    # packed input: [x_q | mask | sel]
    f32 = mybir.dt.float32
    i32 = mybir.dt.int32
    nc = bacc.Bacc(
        "TRN2", target_bir_lowering=False, debug=False, num_devices=CORES
    )

    xall_ext = nc.dram_tensor("xall", [P, W], f32, kind="ExternalInput")
    t_ext = nc.dram_tensor("table", [ROWS * Q, CQ], f32, kind="ExternalInput")
    gidx_ext = nc.dram_tensor("gidx", [P, 1], i32, kind="ExternalInput")
    out_ext = nc.dram_tensor("out", [1, 1], f32, kind="ExternalOutput")

    with tile.TileContext(nc) as tc:
        with (
            tc.tile_pool(name="sbuf", bufs=1) as sb,
            tc.tile_pool(name="psum", bufs=1, space="PSUM") as ps,
        ):
            # warm the ACT exp table while input DMAs are in flight
            warm = sb.tile([1, 1], f32)
            nc.vector.memset(warm[:], 0.0)
            warm2 = sb.tile([1, 1], f32)
            nc.scalar.activation(
                out=warm2[:], in_=warm[:], func=mybir.ActivationFunctionType.Exp
            )

            # ---- loads: gather index first (it gates the indirect DMA chain) ----
            gidx = sb.tile([P, 1], i32)
            nc.sync.dma_start(out=gidx[:], in_=gidx_ext[:])
            xall = sb.tile([P, W], f32)
            nc.sync.dma_start(out=xall[:], in_=xall_ext[:])

            x = xall[:, 0:CQ]
            maskv = xall[0:K, CQ : CQ + 1]           # pre-scaled by -2/N
            sel = xall[:, CQ + 1 : CQ + 1 + K]       # one-hot group matrix

            # ---- gather conf quarter-rows for the owned batch items ----
            conf = sb.tile([P, CQ], f32)
            nc.gpsimd.indirect_dma_start(
                out=conf[:],
                out_offset=None,
                in_=t_ext[:],
                in_offset=bass.IndirectOffsetOnAxis(ap=gidx[:, :1], axis=0),
            )

            # ---- e = exp(x) with per-partition sums (x ~ N(0,1): no shift) ----
            e = sb.tile([P, CQ], f32)
            s4 = sb.tile([P, 1], f32)
            nc.scalar.activation(
                out=e[:],
                in_=x,
                func=mybir.ActivationFunctionType.Exp,
                bias=0.0,
                scale=1.0,
                accum_out=s4[:],
            )

            # ---- group-sum sumexp to K logical rows; w = mask / sumexp ----
            s32p = ps.tile([K, 1], f32)
            nc.tensor.matmul(out=s32p[:], lhsT=sel, rhs=s4[:], start=True, stop=True)
            recip = sb.tile([K, 1], f32)
            nc.vector.reciprocal(out=recip[:], in_=s32p[:])
            w32 = sb.tile([K, 1], f32)
            nc.vector.tensor_mul(w32[:], maskv, recip[:])

            # ---- per-partition rowdot, group-sum, masked total ----
            prod = sb.tile([P, CQ], f32)
            nc.vector.tensor_mul(prod[:], e[:], conf[:])
            r4 = sb.tile([P, 1], f32)
            nc.vector.reduce_sum(out=r4[:], in_=prod[:], axis=mybir.AxisListType.X)
            r32p = ps.tile([K, 1], f32)
            nc.tensor.matmul(out=r32p[:], lhsT=sel, rhs=r4[:], start=True, stop=True)
            r32 = sb.tile([K, 1], f32)
            nc.vector.tensor_copy(out=r32[:], in_=r32p[:])
            acc = ps.tile([1, 1], f32)
            nc.tensor.matmul(out=acc[:], lhsT=r32[:], rhs=w32[:], start=True, stop=True)

            # ---- out_c = partial + 2/CORES ----
            final = sb.tile([1, 1], f32)
            nc.scalar.activation(
                out=final[:],
                in_=acc[:],
                func=mybir.ActivationFunctionType.Copy,
                bias=2.0 / CORES,
                scale=1.0,
            )
            nc.scalar.dma_start(out=out_ext[:], in_=final[:])

    nc.compile()
    return nc


def _get_nc(K):
    if K not in _nc_cache:
        _nc_cache[K] = _build(K)
    return _nc_cache[K]


def kernel(outputs, label_confidence, index):
    global LAST_RESULTS
    outputs = np.ascontiguousarray(np.asarray(outputs, dtype=np.float32))
    label_confidence = np.ascontiguousarray(
        np.asarray(label_confidence, dtype=np.float32)
    )
    idx = np.asarray(index).astype(np.int64).reshape(N)

    owner = idx // ROWS
    counts = np.bincount(owner, minlength=CORES)
    K = 32
    while K < int(counts.max()):
        K *= 2
    Q = P // K
    CQ = C // Q
    W = CQ + 1 + K
    nc = _get_nc(K)

    sel = np.zeros((P, K), dtype=np.float32)
    sel[np.arange(P), np.arange(P) // Q] = 1.0

    in_maps = []
    for c in range(CORES):
        rows = np.nonzero(owner == c)[0]
        n_own = len(rows)
        rows_p = np.concatenate([rows, np.zeros(K - n_own, dtype=rows.dtype)])
        # quarter-expanded gather indices into the [ROWS*Q, CQ] table view
        g = (idx[rows_p] - c * ROWS).astype(np.int64)
        g[n_own:] = 0
        gidx = (g[:, None] * Q + np.arange(Q)[None, :]).astype(np.int32).reshape(P, 1)
        mask = np.full(K, -2.0 / N, dtype=np.float32)
        mask[n_own:] = 0.0

        xall = np.empty((P, W), dtype=np.float32)
        xall[:, 0:CQ] = outputs[rows_p].reshape(P, CQ)
        xall[:, CQ] = 0.0
        xall[0:K, CQ] = mask
        xall[:, CQ + 1 :] = sel
        in_maps.append(
            {
                "xall": xall,
                "table": label_confidence[c * ROWS : (c + 1) * ROWS].reshape(
                    ROWS * Q, CQ
                ),
                "gidx": gidx,
            }
        )
    LAST_RESULTS = run_bass_kernel_spmd(nc, in_maps, core_ids=list(range(CORES)))
    total = np.float32(0.0)
    for c in range(CORES):
        total += np.float32(LAST_RESULTS.results[c]["out"][0, 0])
    return np.asarray(total, dtype=np.float32).reshape(())
